# revision 7
# baseline (speedup 1.0000x reference)
"""Trainium2 Bass kernel v2 for a 2-layer stacked bidirectional LSTM.

Problem (hardcoded): B=64, T=512, D=512, H=512, 2 BiLSTM layers,
Keras gate order [i, f, g, o], sigmoid recurrent activation, tanh cell
activation, merge_mode='concat'.

Sharding: 8 cores = 2 directions x 4 batch quarters (BQ=16 per core).

v2 design (vs v1): ~30 dma_starts instead of ~900, and no DRAM scratch
traffic in the hot loop:
  - Backward cores get time-REVERSED x from the host and produce
    time-reversed h1 (host un-flips). The device program is direction
    agnostic: no per-step selects, no index flips.
  - The input projection is FUSED into the recurrence: per segment of
    TS steps, zx_seg = x_seg @ W + b is computed into SBUF (PE), then
    the TS recurrence steps consume it. No zx DRAM roundtrip.
  - h sequences live in SBUF ([128, KH, T, BQ]); one DMA ships layer-0's
    h to DRAM for the pairwise AllGather; one DMA ships layer-1's h to
    the output.
  - Layer-1 reads its own direction's h from SBUF and the partner
    direction's h from ag_out with a flag-dependent block offset; the
    partner sequence is consumed time-reversed via a negative-stride
    matmul moving-operand access pattern. W1 is host-swapped so
    own-direction rows come first.

Layouts (per core):
  xT     input [D, T*BQ] bf16, token = t_local*BQ + b (bwd: reversed time)
  ag_in  DRAM [128, KH, T, BQ] bf16 (own computation order)
  ag_out DRAM [2, 128, KH, T, BQ] bf16 (group rank order: fwd, bwd)
  h1T    output [128, KH, T, BQ] bf16 (own computation order; host
         converts to [B, T, 2H] f32 and un-flips bwd cores)
"""

import numpy as np
import ml_dtypes

import concourse.bass as bass
import concourse.mybir as mybir
import concourse.tile as tile
from concourse.bass import ds, ts
from concourse.bass_utils import run_bass_kernel_spmd

BF16 = mybir.dt.bfloat16
F32 = mybir.dt.float32
I32 = mybir.dt.int32
AF = mybir.ActivationFunctionType
ALU = mybir.AluOpType

# Problem dims (full size)
B_FULL, T_FULL, D_FULL, H_FULL = 64, 512, 512, 512
N_CORES = 8
N_Q = 4   # batch quarters; cores 2q (fwd) and 2q+1 (bwd) handle quarter q
TS = 16   # recurrence steps per segment (zx staged in SBUF per segment)
CH = 16   # steps per chunk; CH == TS puts all steps inline in the seg body

_MAXW = 1  # max sem-waits per instruction accepted by this walrus


def _fix_walrus_compat(nc):
    """Adapt Tile-emitted IR to the deployed walrus (see kernel v1)."""
    n_split = n_drop = 0
    for bb in nc.main_func.blocks:
        insts = bb.instructions
        out = []
        for inst in insts:
            if isinstance(inst, mybir.InstISA):
                n_drop += 1
                continue
            si = inst.sync_info
            if si is not None and len(si.on_wait) > _MAXW:
                waits = list(si.on_wait)
                extra, keep = waits[:-_MAXW], waits[-_MAXW:]
                for w in extra:
                    nop = mybir.InstNoOp(
                        name=nc.get_next_instruction_name(), ins=[], outs=[])
                    nop.engine = inst.engine
                    nop.sync_info = mybir.SyncInfo(on_wait=[w], on_update=[])
                    out.append(nop)
                    n_split += 1
                inst.sync_info = mybir.SyncInfo(
                    on_wait=keep, on_update=list(si.on_update))
            out.append(inst)
        insts[:] = out
    return n_drop, n_split


def build_program(T=T_FULL, BQ=B_FULL // N_Q, D=D_FULL, H=H_FULL,
                  single_core=False, repeat=1):
    """Build the SPMD bass program (identical for all 8 cores).

    single_core=True replaces the AllGather with local DMA copies (for
    TimelineSim analysis — partner data is wrong but timing-equivalent).
    repeat>1 re-runs each layer's segment loop `repeat` times (timing
    amplification; identical recomputation, output stays correct).
    """
    G = 4 * H
    KD0 = D // 128           # k-chunks layer-0 projection
    KH = H // 128            # k-chunks recurrence
    MC = G // 128            # m-chunks of gate dim
    MG = MC // 4             # m-chunks per gate
    TOK = T * BQ
    TOKSEG = TS * BQ         # tokens per segment
    NSEG = T // TS
    NCH = TS // CH
    assert T % TS == 0 and TS % CH == 0

    nc = bass.Bass("TRN2", target_bir_lowering=False, debug=False,
                   num_devices=1 if single_core else N_CORES)

    # ---- I/O ----
    xT = nc.dram_tensor("xT", [D, TOK], BF16, kind="ExternalInput")
    w0 = nc.dram_tensor("w0", [D, G], BF16, kind="ExternalInput")
    u0 = nc.dram_tensor("u0", [H, G], BF16, kind="ExternalInput")
    b0 = nc.dram_tensor("b0", [G], F32, kind="ExternalInput")
    w1 = nc.dram_tensor("w1", [2 * H, G], BF16, kind="ExternalInput")
    u1 = nc.dram_tensor("u1", [H, G], BF16, kind="ExternalInput")
    b1 = nc.dram_tensor("b1", [G], F32, kind="ExternalInput")
    flag = nc.dram_tensor("flag", [1, 1], I32, kind="ExternalInput")
    ident = nc.dram_tensor("ident", [128, 128], BF16, kind="ExternalInput")
    h1T = nc.dram_tensor("h1T", [128, KH, T, BQ], BF16,
                         kind="ExternalOutput")

    groups = [[2 * q, 2 * q + 1] for q in range(N_Q)]

    with tile.TileContext(nc) as tc:
        dram = tc.alloc_tile_pool(name="dram", bufs=1, space="DRAM")
        ag_in = dram.tile([128, KH, T, BQ], BF16, name="ag_in")
        ag_out = dram.tile([2, 128, KH, T, BQ], BF16, name="ag_out")

        consts = tc.alloc_tile_pool(name="consts", bufs=1)
        b0_sb = consts.tile([128, MC], F32, name="b0_sb")
        nc.sync.dma_start(b0_sb, b0.ap().rearrange("(m p) -> p m", p=128))
        b1_sb = consts.tile([128, MC], F32, name="b1_sb")
        nc.sync.dma_start(b1_sb, b1.ap().rearrange("(m p) -> p m", p=128))
        flag_sb = consts.tile([1, 1], I32, name="flag_sb")
        nc.sync.dma_start(flag_sb, flag.ap())
        id_sb = consts.tile([128, 128], BF16, name="id_sb")
        nc.sync.dma_start(id_sb, ident.ap())
        fv = nc.values_load(flag_sb[0:1, 0:1], min_val=0, max_val=1)

        seqs = tc.alloc_tile_pool(name="seqs", bufs=1)
        h0_seq = seqs.tile([128, KH, T, BQ], BF16, name="h0_seq")

        from contextlib import nullcontext

        def rep_loop():
            return tc.For_i(0, repeat, 1) if repeat > 1 else nullcontext()

        # ---------------- fused projection + recurrence ----------------
        def layer(u_sb, b_sb, h_seq, seg_prep, proj_mm, layer_id):
            """Segment-pair software pipeline: while segment s's recurrence
            runs, the projection for segment s+1 is interleaved into the
            step stream (one m-tile per step) so the PE fills the
            elementwise-tail bubbles. zx ping-pongs between two buffers."""
            state = tc.alloc_tile_pool(name=f"state{layer_id}", bufs=1)
            c_sb = state.tile([128, KH, BQ], F32, name=f"c{layer_id}")
            # static h ring; slot j holds h of step (seg*TS+j); step j's
            # matmuls read slot j-1 (slot CH-1 = previous segment's last)
            h_ck = state.tile([128, KH, CH, BQ], BF16, name=f"hck{layer_id}")
            zxA = state.tile([128, MC, TOKSEG], BF16, name=f"zxA{layer_id}")
            zxB = state.tile([128, MC, TOKSEG], BF16, name=f"zxB{layer_id}")
            assert MC == CH and NSEG % 2 == 0

            with tc.tile_pool(name=f"pps{layer_id}", bufs=2,
                              space="PSUM") as pps, \
                 tc.tile_pool(name=f"zps{layer_id}", bufs=2,
                              space="PSUM") as zps, \
                 tc.tile_pool(name=f"work{layer_id}", bufs=2) as work, \
                 tc.tile_pool(name=f"agp{layer_id}", bufs=1) as agpool, \
                 rep_loop():

                def emit_segment(seg, zx_read, zx_write, ctx_next):
                    """16 recurrence steps reading zx_read; proj m-tile j of
                    the NEXT segment (into zx_write) rides after step j."""
                    swin = h_seq[:, :, ds(nc.s_assert_within(
                        seg * TS, 0, T - CH), CH), :]
                    for j in range(CH):
                        hprev = h_ck[:, :, (j - 1) % CH, :]
                        z_ps = zps.tile([128, MC, BQ], F32, tag="zps")
                        # seed PSUM with zx via one identity matmul (opens
                        # the accumulation group); activations then read z
                        # straight from PSUM (no DVE add on the chain)
                        nc.tensor.matmul(
                            z_ps, id_sb, zx_read[:, :, ts(j, BQ)],
                            start=True, stop=False)
                        for m in range(MC):
                            for k in range(KH):
                                nc.tensor.matmul(
                                    z_ps[:, m, :],
                                    u_sb[:, k, ts(m, 128)],
                                    hprev[:, k, :],
                                    start=False,
                                    stop=(m == MC - 1 and k == KH - 1))
                        # gates host-permuted to [f, i, g, o] with i/f/o
                        # pre-activations halved on the host, so every gate
                        # is a plain tanh: tf=2f-1, ti=2i-1, tg=g, to=2o-1.
                        # Cell state tracked doubled (CS=2c); host halves the
                        # final output. [f,i,g] in one ACT op; o in a second
                        # (only needed at the end of the chain).
                        g16 = work.tile([128, MC, BQ], F32, tag="g16")
                        nc.scalar.activation(
                            g16[:, 0:3 * MG, :], z_ps[:, 0:3 * MG, :],
                            AF.Tanh)
                        nc.scalar.activation(
                            g16[:, 3 * MG:, :], z_ps[:, 3 * MG:, :],
                            AF.Tanh)
                        # fcS = (tf+1)*CS = 4fc ; ig2 = (ti+1)*g = 2ig
                        fc = work.tile([128, MG, BQ], F32, tag="fc")
                        nc.vector.scalar_tensor_tensor(
                            fc, g16[:, 0:MG, :], 1.0, c_sb,
                            ALU.add, ALU.mult)
                        ig = work.tile([128, MG, BQ], F32, tag="ig")
                        nc.vector.scalar_tensor_tensor(
                            ig, g16[:, MG:2 * MG, :], 1.0,
                            g16[:, 2 * MG:3 * MG, :], ALU.add, ALU.mult)
                        # CS' = 2c' = fcS/2 + ig2
                        nc.vector.scalar_tensor_tensor(
                            c_sb, fc, 0.5, ig, ALU.mult, ALU.add)
                        th = work.tile([128, MG, BQ], F32, tag="th")
                        nc.scalar.activation(th, c_sb, AF.Tanh, scale=0.5)
                        # ring holds 2h = (to+1)*tanh(c)
                        nc.vector.scalar_tensor_tensor(
                            h_ck[:, :, j, :], g16[:, 3 * MG:, :], 1.0,
                            th, ALU.add, ALU.mult)
                        # interleaved projection m-tile for the next segment
                        # (emitted last so the zx copy sits at the back of
                        # the per-step ACT queue, off the critical chain)
                        psum = pps.tile([128, TOKSEG], F32, tag="pps")
                        proj_mm(psum, j, ctx_next)
                        nc.scalar.activation(zx_write[:, j, :], psum,
                                             AF.Identity,
                                             bias=b_sb[:, j:j + 1])
                    # ship the segment into the sequence (1 dynamic AP)
                    nc.scalar.copy(swin, h_ck)

                nc.vector.memset(c_sb, 0.0)
                nc.vector.memset(h_ck, 0.0)
                # prologue: projection of segment 0 into zxA
                ctx0 = seg_prep(0, agpool)
                for m in range(MC):
                    psum = pps.tile([128, TOKSEG], F32, tag="pps")
                    proj_mm(psum, m, ctx0)
                    nc.scalar.activation(zxA[:, m, :], psum, AF.Identity,
                                         bias=b_sb[:, m:m + 1])
                with tc.For_i(0, NSEG // 2, 1,
                              hint_engines=(mybir.EngineType.PE,)) as io:
                    segA = nc.s_assert_within(io * 2, 0, NSEG - 2)
                    segB = nc.s_assert_within(io * 2 + 1, 1, NSEG - 1)
                    from concourse.expressions import smin
                    segC = nc.s_assert_within(
                        smin(io * 2 + 2, NSEG - 1), 0, NSEG - 1)
                    ctxB = seg_prep(segB, agpool)
                    emit_segment(segA, zxA, zxB, ctxB)
                    ctxC = seg_prep(segC, agpool)
                    emit_segment(segB, zxB, zxA, ctxC)
            state.release()

        # ---------------- layer 0 ----------------
        xpool = tc.alloc_tile_pool(name="xpool", bufs=1)
        x_sb = xpool.tile([128, KD0, TOK], BF16, name="x_sb")
        nc.sync.dma_start(x_sb, xT.ap().rearrange("(k p) t -> p k t", p=128))
        w0pool = tc.alloc_tile_pool(name="w0pool", bufs=1)
        w0_sb = w0pool.tile([128, KD0, G], BF16, name="w0_sb")
        nc.sync.dma_start(w0_sb, w0.ap().rearrange("(k p) g -> p k g", p=128))
        u0_sb = w0pool.tile([128, KH, G], BF16, name="u0_sb")
        nc.sync.dma_start(u0_sb, u0.ap().rearrange("(k p) g -> p k g", p=128))

        def prep0(seg, agpool):
            # stage the segment's x slice statically so the 64 matmuls
            # carry no symbolic APs (PE register budget)
            toff = nc.s_assert_within(seg * TOKSEG, 0, TOK - TOKSEG)
            xs = agpool.tile([128, KD0, TOKSEG], BF16, tag="xs")
            nc.vector.tensor_copy(xs, x_sb[:, :, ds(toff, TOKSEG)])
            return xs

        def proj0(psum, m, xs):
            for k in range(KD0):
                nc.tensor.matmul(
                    psum, w0_sb[:, k, ts(m, 128)], xs[:, k, :],
                    start=(k == 0), stop=(k == KD0 - 1))

        layer(u0_sb, b0_sb, h0_seq, prep0, proj0, 0)
        w0pool.release()
        xpool.release()

        # ship layer-0 h to DRAM for the exchange (identity layout)
        nc.sync.dma_start(
            ag_in.rearrange("p k t b -> p (k t b)"),
            h0_seq.rearrange("p k t b -> p (k t b)"))

        h1pool = tc.alloc_tile_pool(name="h1pool", bufs=1)
        h1_seq = h1pool.tile([128, KH, T, BQ], BF16, name="h1_seq")
        w1pool = tc.alloc_tile_pool(name="w1pool", bufs=1)
        w1_sb = w1pool.tile([128, 2 * KH, G], BF16, name="w1_sb")
        nc.sync.dma_start(w1_sb, w1.ap().rearrange("(k p) g -> p k g", p=128))
        u1_sb = w1pool.tile([128, KH, G], BF16, name="u1_sb")
        nc.sync.dma_start(u1_sb, u1.ap().rearrange("(k p) g -> p k g", p=128))

        if single_core:
            nc.sync.dma_start(ag_out[0], ag_in)
            nc.sync.dma_start(ag_out[1], ag_in)
        else:
            nc.gpsimd.collective_compute(
                "AllGather", ALU.bypass, replica_groups=groups,
                ins=[ag_in.opt()], outs=[ag_out.opt()])

        # ---------------- layer 1 ----------------
        pidx = nc.s_assert_within(1 - fv, 0, 1)

        def prep1(seg, agpool):
            # partner's storage segment mirrored in time, loaded straight
            agp = agpool.tile([128, KH, TS, BQ], BF16, tag="agp")
            src = ag_out[ds(pidx, 1), :, :,
                         ds((NSEG - 1) * TS - seg * TS, TS), :]
            nc.sync.dma_start(agp, src.squeeze(0))
            # stage own-direction h statically (PE register budget)
            toff = nc.s_assert_within(seg * TS, 0, T - TS)
            own = agpool.tile([128, KH, TS, BQ], BF16, tag="own")
            nc.vector.tensor_copy(own, h0_seq[:, :, ds(toff, TS), :])
            return (agp, own)

        def proj1(psum, m, ctx):
            agp, own = ctx
            # own-direction features (straight)
            for k in range(KH):
                nc.tensor.matmul(
                    psum, w1_sb[:, k, ts(m, 128)], own[:, k, :, :],
                    start=(k == 0), stop=False)
            # partner features: reversed within the loaded tile
            for k in range(KH):
                nc.tensor.matmul(
                    psum, w1_sb[:, KH + k, ts(m, 128)],
                    agp[:, k, ::-1, :],
                    start=False, stop=(k == KH - 1))

        layer(u1_sb, b1_sb, h1_seq, prep1, proj1, 1)
        w1pool.release()

        nc.sync.dma_start(h1T.ap().rearrange("p k t b -> p (k t b)"),
                          h1_seq.rearrange("p k t b -> p (k t b)"))

        h1pool.release()
        seqs.release()
        consts.release()
        dram.release()

    _fix_walrus_compat(nc)
    return nc


def _prep_core_inputs(x, W0f, U0f, b0f, W0b, U0b, b0b,
                      W1f, U1f, b1f, W1b, U1b, b1b, T, BQ):
    """Host-side sharding: returns list of 8 input dicts (core = 2q+dir).

    Backward cores receive time-reversed x and W1 with row halves swapped
    (own-direction features first)."""
    bf = ml_dtypes.bfloat16
    H = U0f.shape[0]

    def gperm(M, row_scale=1.0):
        # reorder gate blocks [i, f, g, o] -> [f, i, g, o] on the last axis,
        # halve the f/i/o pre-activations (sigmoid(x) = (tanh(x/2)+1)/2 so
        # the device computes every gate as a plain tanh), and apply
        # row_scale (0.5 for matrices consuming the doubled h convention).
        M = np.asarray(M, dtype=np.float32)
        M4 = M.reshape(*M.shape[:-1], 4, H)[..., [1, 0, 2, 3], :]
        cs = np.array([0.5, 0.5, 1.0, 0.5], dtype=np.float32)
        M4 = M4 * cs[:, None] * np.float32(row_scale)
        return np.ascontiguousarray(M4.reshape(M.shape))

    in_maps = []
    Wd = {0: (W0f, U0f, b0f, W1f, U1f, b1f),
          1: (W0b, U0b, b0b, W1b, U1b, b1b)}
    for q in range(N_Q):
        xq = x[q * BQ:(q + 1) * BQ]              # [BQ, T, D]
        for d in range(2):
            W0, U0, b0, W1, U1, b1 = Wd[d]
            xd = xq if d == 0 else xq[:, ::-1]
            xTc = np.ascontiguousarray(
                xd.transpose(2, 1, 0).reshape(x.shape[2], T * BQ)).astype(bf)
            W1c = W1 if d == 0 else np.concatenate([W1[H:], W1[:H]], axis=0)
            in_maps.append({
                "xT": xTc,
                "w0": gperm(W0).astype(bf),
                "u0": gperm(U0, 0.5).astype(bf),
                "b0": gperm(b0).astype(np.float32),
                "w1": gperm(W1c, 0.5).astype(bf),
                "u1": gperm(U1, 0.5).astype(bf),
                "b1": gperm(b1).astype(np.float32),
                "flag": np.array([[d]], dtype=np.int32),
                "ident": np.eye(128, dtype=np.float32).astype(bf),
            })
    return in_maps


def _unshard(results, B, T, H, BQ):
    out = np.empty((B, T, 2 * H), dtype=np.float32)
    KH = H // 128
    for q in range(N_Q):
        for d in range(2):
            h1 = np.asarray(results[2 * q + d]["h1T"], dtype=np.float32)
            # h1 [128, KH, T, BQ] holds 2*h (doubled-h convention)
            h = h1.transpose(3, 2, 1, 0).reshape(BQ, T, H) * np.float32(0.5)
            if d == 1:
                h = h[:, ::-1]
            out[q * BQ:(q + 1) * BQ, :, d * H:(d + 1) * H] = h
    return out


def kernel(x, W0f, U0f, b0f, W0b, U0b, b0b,
           W1f, U1f, b1f, W1b, U1b, b1b):
    x = np.asarray(x, dtype=np.float32)
    B, T, D = x.shape
    H = U0f.shape[0]
    BQ = B // N_Q
    nc = build_program(T=T, BQ=BQ, D=D, H=H)
    in_maps = _prep_core_inputs(
        np.asarray(x), np.asarray(W0f), np.asarray(U0f), np.asarray(b0f),
        np.asarray(W0b), np.asarray(U0b), np.asarray(b0b),
        np.asarray(W1f), np.asarray(U1f), np.asarray(b1f),
        np.asarray(W1b), np.asarray(U1b), np.asarray(b1b), T, BQ)
    res = run_bass_kernel_spmd(nc, in_maps, list(range(N_CORES)))
    return _unshard(res.results, B, T, H, BQ)



# revision 10
# speedup vs baseline: 1.0093x; 1.0093x over previous
"""Trainium2 Bass kernel v2 for a 2-layer stacked bidirectional LSTM.

Problem (hardcoded): B=64, T=512, D=512, H=512, 2 BiLSTM layers,
Keras gate order [i, f, g, o], sigmoid recurrent activation, tanh cell
activation, merge_mode='concat'.

Sharding: 8 cores = 2 directions x 4 batch quarters (BQ=16 per core).

v2 design (vs v1): ~30 dma_starts instead of ~900, and no DRAM scratch
traffic in the hot loop:
  - Backward cores get time-REVERSED x from the host and produce
    time-reversed h1 (host un-flips). The device program is direction
    agnostic: no per-step selects, no index flips.
  - The input projection is FUSED into the recurrence: per segment of
    TS steps, zx_seg = x_seg @ W + b is computed into SBUF (PE), then
    the TS recurrence steps consume it. No zx DRAM roundtrip.
  - h sequences live in SBUF ([128, KH, T, BQ]); one DMA ships layer-0's
    h to DRAM for the pairwise AllGather; one DMA ships layer-1's h to
    the output.
  - Layer-1 reads its own direction's h from SBUF and the partner
    direction's h from ag_out with a flag-dependent block offset; the
    partner sequence is consumed time-reversed via a negative-stride
    matmul moving-operand access pattern. W1 is host-swapped so
    own-direction rows come first.

Layouts (per core):
  xT     input [D, T*BQ] bf16, token = t_local*BQ + b (bwd: reversed time)
  ag_in  DRAM [128, KH, T, BQ] bf16 (own computation order)
  ag_out DRAM [2, 128, KH, T, BQ] bf16 (group rank order: fwd, bwd)
  h1T    output [128, KH, T, BQ] bf16 (own computation order; host
         converts to [B, T, 2H] f32 and un-flips bwd cores)
"""

import numpy as np
import ml_dtypes

import concourse.bass as bass
import concourse.mybir as mybir
import concourse.tile as tile
from concourse.bass import ds, ts
from concourse.bass_utils import run_bass_kernel_spmd

BF16 = mybir.dt.bfloat16
F32 = mybir.dt.float32
I32 = mybir.dt.int32
AF = mybir.ActivationFunctionType
ALU = mybir.AluOpType

# Problem dims (full size)
B_FULL, T_FULL, D_FULL, H_FULL = 64, 512, 512, 512
N_CORES = 8
N_Q = 4   # batch quarters; cores 2q (fwd) and 2q+1 (bwd) handle quarter q
TS = 16   # recurrence steps per segment (zx staged in SBUF per segment)
CH = 16   # steps per chunk; CH == TS puts all steps inline in the seg body

_MAXW = 1  # max sem-waits per instruction accepted by this walrus


def _fix_walrus_compat(nc):
    """Adapt Tile-emitted IR to the deployed walrus (see kernel v1)."""
    n_split = n_drop = 0
    for bb in nc.main_func.blocks:
        insts = bb.instructions
        out = []
        for inst in insts:
            if isinstance(inst, mybir.InstISA):
                n_drop += 1
                continue
            si = inst.sync_info
            if si is not None and len(si.on_wait) > _MAXW:
                waits = list(si.on_wait)
                extra, keep = waits[:-_MAXW], waits[-_MAXW:]
                for w in extra:
                    nop = mybir.InstNoOp(
                        name=nc.get_next_instruction_name(), ins=[], outs=[])
                    nop.engine = inst.engine
                    nop.sync_info = mybir.SyncInfo(on_wait=[w], on_update=[])
                    out.append(nop)
                    n_split += 1
                inst.sync_info = mybir.SyncInfo(
                    on_wait=keep, on_update=list(si.on_update))
            out.append(inst)
        insts[:] = out
    return n_drop, n_split


def build_program(T=T_FULL, BQ=B_FULL // N_Q, D=D_FULL, H=H_FULL,
                  single_core=False, repeat=1):
    """Build the SPMD bass program (identical for all 8 cores).

    single_core=True replaces the AllGather with local DMA copies (for
    TimelineSim analysis — partner data is wrong but timing-equivalent).
    repeat>1 re-runs each layer's segment loop `repeat` times (timing
    amplification; identical recomputation, output stays correct).
    """
    G = 4 * H
    KD0 = D // 128           # k-chunks layer-0 projection
    KH = H // 128            # k-chunks recurrence
    MC = G // 128            # m-chunks of gate dim
    MG = MC // 4             # m-chunks per gate
    TOK = T * BQ
    TOKSEG = TS * BQ         # tokens per segment
    NSEG = T // TS
    NCH = TS // CH
    assert T % TS == 0 and TS % CH == 0

    nc = bass.Bass("TRN2", target_bir_lowering=False, debug=False,
                   num_devices=1 if single_core else N_CORES)

    # ---- I/O ----
    xT = nc.dram_tensor("xT", [D, TOK], BF16, kind="ExternalInput")
    w0 = nc.dram_tensor("w0", [D, G], BF16, kind="ExternalInput")
    u0 = nc.dram_tensor("u0", [H, G], BF16, kind="ExternalInput")
    b0 = nc.dram_tensor("b0", [G], F32, kind="ExternalInput")
    w1 = nc.dram_tensor("w1", [2 * H, G], BF16, kind="ExternalInput")
    u1 = nc.dram_tensor("u1", [H, G], BF16, kind="ExternalInput")
    b1 = nc.dram_tensor("b1", [G], F32, kind="ExternalInput")
    flag = nc.dram_tensor("flag", [1, 1], I32, kind="ExternalInput")
    ident = nc.dram_tensor("ident", [128, 128], BF16, kind="ExternalInput")
    h1T = nc.dram_tensor("h1T", [128, KH, T, BQ], BF16,
                         kind="ExternalOutput")

    groups = [[2 * q, 2 * q + 1] for q in range(N_Q)]

    with tile.TileContext(nc) as tc:
        dram = tc.alloc_tile_pool(name="dram", bufs=1, space="DRAM")
        ag_in = dram.tile([128, KH, T, BQ], BF16, name="ag_in")
        ag_out = dram.tile([2, 128, KH, T, BQ], BF16, name="ag_out")

        consts = tc.alloc_tile_pool(name="consts", bufs=1)
        b0_sb = consts.tile([128, MC], F32, name="b0_sb")
        nc.sync.dma_start(b0_sb, b0.ap().rearrange("(m p) -> p m", p=128))
        b1_sb = consts.tile([128, MC], F32, name="b1_sb")
        nc.sync.dma_start(b1_sb, b1.ap().rearrange("(m p) -> p m", p=128))
        flag_sb = consts.tile([1, 1], I32, name="flag_sb")
        nc.sync.dma_start(flag_sb, flag.ap())
        id_sb = consts.tile([128, 128], BF16, name="id_sb")
        nc.sync.dma_start(id_sb, ident.ap())
        fv = nc.values_load(flag_sb[0:1, 0:1], min_val=0, max_val=1)

        seqs = tc.alloc_tile_pool(name="seqs", bufs=1)
        h0_seq = seqs.tile([128, KH, T, BQ], BF16, name="h0_seq")

        from contextlib import nullcontext

        def rep_loop():
            return tc.For_i(0, repeat, 1) if repeat > 1 else nullcontext()

        # ---------------- fused projection + recurrence ----------------
        def layer(u_sb, b_sb, h_seq, seg_prep, proj_mm, layer_id):
            """Segment-pair software pipeline: while segment s's recurrence
            runs, the projection for segment s+1 is interleaved into the
            step stream (one m-tile per step) so the PE fills the
            elementwise-tail bubbles. zx ping-pongs between two buffers."""
            state = tc.alloc_tile_pool(name=f"state{layer_id}", bufs=1)
            c_sb = state.tile([128, KH, BQ], F32, name=f"c{layer_id}")
            # static h ring; slot j holds h of step (seg*TS+j); step j's
            # matmuls read slot j-1 (slot CH-1 = previous segment's last)
            h_ck = state.tile([128, KH, CH, BQ], BF16, name=f"hck{layer_id}")
            zxA = state.tile([128, MC, TOKSEG], BF16, name=f"zxA{layer_id}")
            zxB = state.tile([128, MC, TOKSEG], BF16, name=f"zxB{layer_id}")
            assert MC == CH and NSEG % 2 == 0

            with tc.tile_pool(name=f"pps{layer_id}", bufs=2,
                              space="PSUM") as pps, \
                 tc.tile_pool(name=f"zps{layer_id}", bufs=2,
                              space="PSUM") as zps, \
                 tc.tile_pool(name=f"work{layer_id}", bufs=2) as work, \
                 tc.tile_pool(name=f"agp{layer_id}", bufs=1) as agpool, \
                 rep_loop():

                def emit_segment(seg, zx_read, zx_write, ctx_next):
                    """16 recurrence steps reading zx_read; proj m-tile j of
                    the NEXT segment (into zx_write) rides after step j."""
                    swin = h_seq[:, :, ds(nc.s_assert_within(
                        seg * TS, 0, T - CH), CH), :]
                    for j in range(CH):
                        hprev = h_ck[:, :, (j - 1) % CH, :]
                        # z viewed [128, gate, k, b]; m-tile = gate*4 + k
                        z_ps = zps.tile([128, 4, KH, BQ], F32, tag="zps")
                        # seed PSUM with zx via one identity matmul (opens
                        # the accumulation group); activations then read z
                        # straight from PSUM (no DVE add on the chain)
                        nc.tensor.matmul(
                            z_ps, id_sb, zx_read[:, :, ts(j, BQ)],
                            start=True, stop=False)
                        # k_in-major waves: wave k only needs h-chunk k of
                        # the previous step, which is produced in halves, so
                        # waves 0-1 start as soon as h-half0 lands. Within
                        # the last wave, the [f,i,g]-half0 tiles come first
                        # so the first gate ACT fires earliest.
                        worder = ([g * 4 + ko for g in (0, 1, 2)
                                   for ko in (0, 1)]
                                  + [g * 4 + ko for g in (0, 1, 2)
                                     for ko in (2, 3)]
                                  + [12, 13, 14, 15])
                        for k in range(KH):
                            for m in worder:
                                # stop only on the very last mm: a stop
                                # closes the whole 2KB psum zero-region, so
                                # per-tile stops would orphan later
                                # accumulations in the same bank.
                                nc.tensor.matmul(
                                    z_ps[:, m // 4, m % 4, :],
                                    u_sb[:, k, ts(m, 128)],
                                    hprev[:, k, :],
                                    start=False,
                                    stop=(k == KH - 1 and m == worder[-1]))
                        # gates host-permuted to [f, i, g, o] with i/f/o
                        # pre-activations halved on the host, so every gate
                        # is a plain tanh: tf=2f-1, ti=2i-1, tg=g, to=2o-1.
                        # Cell state tracked doubled (CS=2c); host halves the
                        # final output. Tail split into k-halves so h-half0
                        # unblocks the next step's first waves early.
                        g16 = work.tile([128, 4, KH, BQ], F32, tag="g16")
                        nc.scalar.activation(
                            g16[:, 0:3, :, :], z_ps[:, 0:3, :, :], AF.Tanh)
                        nc.scalar.activation(
                            g16[:, 3, :, :], z_ps[:, 3, :, :], AF.Tanh)
                        # fcS = (tf+1)*CS = 4fc ; ig2 = (ti+1)*g = 2ig
                        fc = work.tile([128, KH, BQ], F32, tag="fc")
                        nc.vector.scalar_tensor_tensor(
                            fc, g16[:, 0, :, :], 1.0, c_sb,
                            ALU.add, ALU.mult)
                        ig = work.tile([128, KH, BQ], F32, tag="ig")
                        nc.vector.scalar_tensor_tensor(
                            ig, g16[:, 1, :, :], 1.0, g16[:, 2, :, :],
                            ALU.add, ALU.mult)
                        # CS' = 2c' = fcS/2 + ig2
                        nc.vector.scalar_tensor_tensor(
                            c_sb, fc, 0.5, ig, ALU.mult, ALU.add)
                        th = work.tile([128, KH, BQ], F32, tag="th")
                        nc.scalar.activation(th, c_sb, AF.Tanh, scale=0.5)
                        # ring holds 2h = (to+1)*tanh(c)
                        nc.vector.scalar_tensor_tensor(
                            h_ck[:, :, j, :], g16[:, 3, :, :], 1.0,
                            th, ALU.add, ALU.mult)
                        # interleaved projection m-tile for the next segment
                        # (emitted last so the zx copy sits at the back of
                        # the per-step ACT queue, off the critical chain)
                        psum = pps.tile([128, TOKSEG], F32, tag="pps")
                        proj_mm(psum, j, ctx_next)
                        nc.scalar.activation(zx_write[:, j, :], psum,
                                             AF.Identity,
                                             bias=b_sb[:, j:j + 1])
                    # ship the segment into the sequence (1 dynamic AP)
                    nc.scalar.copy(swin, h_ck)

                nc.vector.memset(c_sb, 0.0)
                nc.vector.memset(h_ck, 0.0)
                # prologue: projection of segment 0 into zxA
                ctx0 = seg_prep(0, agpool)
                for m in range(MC):
                    psum = pps.tile([128, TOKSEG], F32, tag="pps")
                    proj_mm(psum, m, ctx0)
                    nc.scalar.activation(zxA[:, m, :], psum, AF.Identity,
                                         bias=b_sb[:, m:m + 1])
                with tc.For_i(0, NSEG // 2, 1,
                              hint_engines=(mybir.EngineType.PE,)) as io:
                    segA = nc.s_assert_within(io * 2, 0, NSEG - 2)
                    segB = nc.s_assert_within(io * 2 + 1, 1, NSEG - 1)
                    from concourse.expressions import smin
                    segC = nc.s_assert_within(
                        smin(io * 2 + 2, NSEG - 1), 0, NSEG - 1)
                    ctxB = seg_prep(segB, agpool)
                    emit_segment(segA, zxA, zxB, ctxB)
                    ctxC = seg_prep(segC, agpool)
                    emit_segment(segB, zxB, zxA, ctxC)
            state.release()

        # ---------------- layer 0 ----------------
        xpool = tc.alloc_tile_pool(name="xpool", bufs=1)
        x_sb = xpool.tile([128, KD0, TOK], BF16, name="x_sb")
        nc.sync.dma_start(x_sb, xT.ap().rearrange("(k p) t -> p k t", p=128))
        w0pool = tc.alloc_tile_pool(name="w0pool", bufs=1)
        w0_sb = w0pool.tile([128, KD0, G], BF16, name="w0_sb")
        nc.sync.dma_start(w0_sb, w0.ap().rearrange("(k p) g -> p k g", p=128))
        u0_sb = w0pool.tile([128, KH, G], BF16, name="u0_sb")
        nc.sync.dma_start(u0_sb, u0.ap().rearrange("(k p) g -> p k g", p=128))

        def prep0(seg, agpool):
            # stage the segment's x slice statically so the 64 matmuls
            # carry no symbolic APs (PE register budget)
            toff = nc.s_assert_within(seg * TOKSEG, 0, TOK - TOKSEG)
            xs = agpool.tile([128, KD0, TOKSEG], BF16, tag="xs")
            nc.vector.tensor_copy(xs, x_sb[:, :, ds(toff, TOKSEG)])
            return xs

        def proj0(psum, m, xs):
            for k in range(KD0):
                nc.tensor.matmul(
                    psum, w0_sb[:, k, ts(m, 128)], xs[:, k, :],
                    start=(k == 0), stop=(k == KD0 - 1))

        layer(u0_sb, b0_sb, h0_seq, prep0, proj0, 0)
        w0pool.release()
        xpool.release()

        # ship layer-0 h to DRAM for the exchange (identity layout)
        nc.sync.dma_start(
            ag_in.rearrange("p k t b -> p (k t b)"),
            h0_seq.rearrange("p k t b -> p (k t b)"))

        h1pool = tc.alloc_tile_pool(name="h1pool", bufs=1)
        h1_seq = h1pool.tile([128, KH, T, BQ], BF16, name="h1_seq")
        w1pool = tc.alloc_tile_pool(name="w1pool", bufs=1)
        w1_sb = w1pool.tile([128, 2 * KH, G], BF16, name="w1_sb")
        nc.sync.dma_start(w1_sb, w1.ap().rearrange("(k p) g -> p k g", p=128))
        u1_sb = w1pool.tile([128, KH, G], BF16, name="u1_sb")
        nc.sync.dma_start(u1_sb, u1.ap().rearrange("(k p) g -> p k g", p=128))

        if single_core:
            nc.sync.dma_start(ag_out[0], ag_in)
            nc.sync.dma_start(ag_out[1], ag_in)
        else:
            nc.gpsimd.collective_compute(
                "AllGather", ALU.bypass, replica_groups=groups,
                ins=[ag_in.opt()], outs=[ag_out.opt()])

        # ---------------- layer 1 ----------------
        pidx = nc.s_assert_within(1 - fv, 0, 1)

        def prep1(seg, agpool):
            # partner's storage segment mirrored in time, loaded straight
            agp = agpool.tile([128, KH, TS, BQ], BF16, tag="agp")
            src = ag_out[ds(pidx, 1), :, :,
                         ds((NSEG - 1) * TS - seg * TS, TS), :]
            nc.sync.dma_start(agp, src.squeeze(0))
            # stage own-direction h statically (PE register budget)
            toff = nc.s_assert_within(seg * TS, 0, T - TS)
            own = agpool.tile([128, KH, TS, BQ], BF16, tag="own")
            nc.vector.tensor_copy(own, h0_seq[:, :, ds(toff, TS), :])
            return (agp, own)

        def proj1(psum, m, ctx):
            agp, own = ctx
            # own-direction features (straight)
            for k in range(KH):
                nc.tensor.matmul(
                    psum, w1_sb[:, k, ts(m, 128)], own[:, k, :, :],
                    start=(k == 0), stop=False)
            # partner features: reversed within the loaded tile
            for k in range(KH):
                nc.tensor.matmul(
                    psum, w1_sb[:, KH + k, ts(m, 128)],
                    agp[:, k, ::-1, :],
                    start=False, stop=(k == KH - 1))

        layer(u1_sb, b1_sb, h1_seq, prep1, proj1, 1)
        w1pool.release()

        nc.sync.dma_start(h1T.ap().rearrange("p k t b -> p (k t b)"),
                          h1_seq.rearrange("p k t b -> p (k t b)"))

        h1pool.release()
        seqs.release()
        consts.release()
        dram.release()

    _fix_walrus_compat(nc)
    return nc


def _prep_core_inputs(x, W0f, U0f, b0f, W0b, U0b, b0b,
                      W1f, U1f, b1f, W1b, U1b, b1b, T, BQ):
    """Host-side sharding: returns list of 8 input dicts (core = 2q+dir).

    Backward cores receive time-reversed x and W1 with row halves swapped
    (own-direction features first)."""
    bf = ml_dtypes.bfloat16
    H = U0f.shape[0]

    def gperm(M, row_scale=1.0):
        # reorder gate blocks [i, f, g, o] -> [f, i, g, o] on the last axis,
        # halve the f/i/o pre-activations (sigmoid(x) = (tanh(x/2)+1)/2 so
        # the device computes every gate as a plain tanh), and apply
        # row_scale (0.5 for matrices consuming the doubled h convention).
        M = np.asarray(M, dtype=np.float32)
        M4 = M.reshape(*M.shape[:-1], 4, H)[..., [1, 0, 2, 3], :]
        cs = np.array([0.5, 0.5, 1.0, 0.5], dtype=np.float32)
        M4 = M4 * cs[:, None] * np.float32(row_scale)
        return np.ascontiguousarray(M4.reshape(M.shape))

    in_maps = []
    Wd = {0: (W0f, U0f, b0f, W1f, U1f, b1f),
          1: (W0b, U0b, b0b, W1b, U1b, b1b)}
    for q in range(N_Q):
        xq = x[q * BQ:(q + 1) * BQ]              # [BQ, T, D]
        for d in range(2):
            W0, U0, b0, W1, U1, b1 = Wd[d]
            xd = xq if d == 0 else xq[:, ::-1]
            xTc = np.ascontiguousarray(
                xd.transpose(2, 1, 0).reshape(x.shape[2], T * BQ)).astype(bf)
            W1c = W1 if d == 0 else np.concatenate([W1[H:], W1[:H]], axis=0)
            in_maps.append({
                "xT": xTc,
                "w0": gperm(W0).astype(bf),
                "u0": gperm(U0, 0.5).astype(bf),
                "b0": gperm(b0).astype(np.float32),
                "w1": gperm(W1c, 0.5).astype(bf),
                "u1": gperm(U1, 0.5).astype(bf),
                "b1": gperm(b1).astype(np.float32),
                "flag": np.array([[d]], dtype=np.int32),
                "ident": np.eye(128, dtype=np.float32).astype(bf),
            })
    return in_maps


def _unshard(results, B, T, H, BQ):
    out = np.empty((B, T, 2 * H), dtype=np.float32)
    KH = H // 128
    for q in range(N_Q):
        for d in range(2):
            h1 = np.asarray(results[2 * q + d]["h1T"], dtype=np.float32)
            # h1 [128, KH, T, BQ] holds 2*h (doubled-h convention)
            h = h1.transpose(3, 2, 1, 0).reshape(BQ, T, H) * np.float32(0.5)
            if d == 1:
                h = h[:, ::-1]
            out[q * BQ:(q + 1) * BQ, :, d * H:(d + 1) * H] = h
    return out


def kernel(x, W0f, U0f, b0f, W0b, U0b, b0b,
           W1f, U1f, b1f, W1b, U1b, b1b):
    x = np.asarray(x, dtype=np.float32)
    B, T, D = x.shape
    H = U0f.shape[0]
    BQ = B // N_Q
    nc = build_program(T=T, BQ=BQ, D=D, H=H)
    in_maps = _prep_core_inputs(
        np.asarray(x), np.asarray(W0f), np.asarray(U0f), np.asarray(b0f),
        np.asarray(W0b), np.asarray(U0b), np.asarray(b0b),
        np.asarray(W1f), np.asarray(U1f), np.asarray(b1f),
        np.asarray(W1b), np.asarray(U1b), np.asarray(b1b), T, BQ)
    res = run_bass_kernel_spmd(nc, in_maps, list(range(N_CORES)))
    return _unshard(res.results, B, T, H, BQ)



# revision 26
# speedup vs baseline: 1.0325x; 1.0230x over previous
"""Trainium2 Bass kernel v2 for a 2-layer stacked bidirectional LSTM.

Problem (hardcoded): B=64, T=512, D=512, H=512, 2 BiLSTM layers,
Keras gate order [i, f, g, o], sigmoid recurrent activation, tanh cell
activation, merge_mode='concat'.

Sharding: 8 cores = 2 directions x 4 batch quarters (BQ=16 per core).

v2 design (vs v1): ~30 dma_starts instead of ~900, and no DRAM scratch
traffic in the hot loop:
  - Backward cores get time-REVERSED x from the host and produce
    time-reversed h1 (host un-flips). The device program is direction
    agnostic: no per-step selects, no index flips.
  - The input projection is FUSED into the recurrence: per segment of
    TS steps, zx_seg = x_seg @ W + b is computed into SBUF (PE), then
    the TS recurrence steps consume it. No zx DRAM roundtrip.
  - h sequences live in SBUF ([128, KH, T, BQ]); one DMA ships layer-0's
    h to DRAM for the pairwise AllGather; one DMA ships layer-1's h to
    the output.
  - Layer-1 reads its own direction's h from SBUF and the partner
    direction's h from ag_out with a flag-dependent block offset; the
    partner sequence is consumed time-reversed via a negative-stride
    matmul moving-operand access pattern. W1 is host-swapped so
    own-direction rows come first.

Layouts (per core):
  xT     input [D, T*BQ] bf16, token = t_local*BQ + b (bwd: reversed time)
  ag_in  DRAM [128, KH, T, BQ] bf16 (own computation order)
  ag_out DRAM [2, 128, KH, T, BQ] bf16 (group rank order: fwd, bwd)
  h1T    output [128, KH, T, BQ] bf16 (own computation order; host
         converts to [B, T, 2H] f32 and un-flips bwd cores)
"""

import numpy as np
import ml_dtypes

import concourse.bass as bass
import concourse.mybir as mybir
import concourse.tile as tile
from concourse.bass import ds, ts
from concourse.bass_utils import run_bass_kernel_spmd

BF16 = mybir.dt.bfloat16
F32 = mybir.dt.float32
I32 = mybir.dt.int32
AF = mybir.ActivationFunctionType
ALU = mybir.AluOpType

# Problem dims (full size)
B_FULL, T_FULL, D_FULL, H_FULL = 64, 512, 512, 512
N_CORES = 8
N_Q = 4   # batch quarters; cores 2q (fwd) and 2q+1 (bwd) handle quarter q
TS = 16   # recurrence steps per segment (zx staged in SBUF per segment)
CH = 16   # steps per chunk; CH == TS puts all steps inline in the seg body

_MAXW = 1  # max sem-waits per instruction accepted by this walrus


def _fix_walrus_compat(nc):
    """Adapt Tile-emitted IR to the deployed walrus (see kernel v1)."""
    n_split = n_drop = 0
    for bb in nc.main_func.blocks:
        insts = bb.instructions
        out = []
        for inst in insts:
            if isinstance(inst, mybir.InstISA):
                n_drop += 1
                continue
            si = inst.sync_info
            if si is not None and len(si.on_wait) > _MAXW:
                waits = list(si.on_wait)
                extra, keep = waits[:-_MAXW], waits[-_MAXW:]
                for w in extra:
                    nop = mybir.InstNoOp(
                        name=nc.get_next_instruction_name(), ins=[], outs=[])
                    nop.engine = inst.engine
                    nop.sync_info = mybir.SyncInfo(on_wait=[w], on_update=[])
                    out.append(nop)
                    n_split += 1
                inst.sync_info = mybir.SyncInfo(
                    on_wait=keep, on_update=list(si.on_update))
            out.append(inst)
        insts[:] = out
    return n_drop, n_split


def build_program(T=T_FULL, BQ=B_FULL // N_Q, D=D_FULL, H=H_FULL,
                  single_core=False, repeat=1):
    """Build the SPMD bass program (identical for all 8 cores).

    single_core=True replaces the AllGather with local DMA copies (for
    TimelineSim analysis — partner data is wrong but timing-equivalent).
    repeat>1 re-runs each layer's segment loop `repeat` times (timing
    amplification; identical recomputation, output stays correct).
    """
    G = 4 * H
    KD0 = D // 128           # k-chunks layer-0 projection
    KH = H // 128            # k-chunks recurrence
    MC = G // 128            # m-chunks of gate dim
    MG = MC // 4             # m-chunks per gate
    TOK = T * BQ
    TOKSEG = TS * BQ         # tokens per segment
    NSEG = T // TS
    NCH = TS // CH
    assert T % TS == 0 and TS % CH == 0

    nc = bass.Bass("TRN2", target_bir_lowering=False, debug=False,
                   num_devices=1 if single_core else N_CORES)

    # ---- I/O ----
    xT = nc.dram_tensor("xT", [D, TOK], BF16, kind="ExternalInput")
    w0 = nc.dram_tensor("w0", [D, G], BF16, kind="ExternalInput")
    u0 = nc.dram_tensor("u0", [H, G], BF16, kind="ExternalInput")
    b0 = nc.dram_tensor("b0", [G], F32, kind="ExternalInput")
    w1 = nc.dram_tensor("w1", [2 * H, G], BF16, kind="ExternalInput")
    u1 = nc.dram_tensor("u1", [H, G], BF16, kind="ExternalInput")
    b1 = nc.dram_tensor("b1", [G], F32, kind="ExternalInput")
    flag = nc.dram_tensor("flag", [1, 1], I32, kind="ExternalInput")
    ident = nc.dram_tensor("ident", [128, 128], BF16, kind="ExternalInput")
    h1T = nc.dram_tensor("h1T", [128, KH, T, BQ], BF16,
                         kind="ExternalOutput")

    groups = [[2 * q, 2 * q + 1] for q in range(N_Q)]

    with tile.TileContext(nc) as tc:
        dram = tc.alloc_tile_pool(name="dram", bufs=1, space="DRAM")
        ag_in = dram.tile([128, KH, T, BQ], BF16, name="ag_in")
        ag_out = dram.tile([2, 128, KH, T, BQ], BF16, name="ag_out")

        consts = tc.alloc_tile_pool(name="consts", bufs=1)
        b0_sb = consts.tile([128, MC], F32, name="b0_sb")
        nc.sync.dma_start(b0_sb, b0.ap().rearrange("(m p) -> p m", p=128))
        b1_sb = consts.tile([128, MC], F32, name="b1_sb")
        nc.sync.dma_start(b1_sb, b1.ap().rearrange("(m p) -> p m", p=128))
        flag_sb = consts.tile([1, 1], I32, name="flag_sb")
        nc.sync.dma_start(flag_sb, flag.ap())
        id_sb = consts.tile([128, 128], BF16, name="id_sb")
        nc.sync.dma_start(id_sb, ident.ap())
        fv = nc.values_load(flag_sb[0:1, 0:1], min_val=0, max_val=1)

        seqs = tc.alloc_tile_pool(name="seqs", bufs=1)
        h0_seq = seqs.tile([128, KH, T, BQ], BF16, name="h0_seq")

        from contextlib import nullcontext

        def rep_loop():
            return tc.For_i(0, repeat, 1) if repeat > 1 else nullcontext()

        # ---------------- fused projection + recurrence ----------------
        def layer(u_sb, b_sb, ship, seg_prep, proj_mm, layer_id):
            """Segment-pair software pipeline: while segment s's recurrence
            runs, the projection for segment s+1 is interleaved into the
            step stream (one m-tile per step) so the PE fills the
            elementwise-tail bubbles. zx ping-pongs between two buffers."""
            state = tc.alloc_tile_pool(name=f"state{layer_id}", bufs=1)
            c_sb = state.tile([128, KH, BQ], F32, name=f"c{layer_id}")
            # static h ring; slot j holds h of step (seg*TS+j); step j's
            # matmuls read slot j-1 (slot CH-1 = previous segment's last)
            h_ck = state.tile([128, KH, CH, BQ], BF16, name=f"hck{layer_id}")
            zxA = state.tile([128, MC, TOKSEG], BF16, name=f"zxA{layer_id}")
            zxB = state.tile([128, MC, TOKSEG], BF16, name=f"zxB{layer_id}")
            assert MC == CH and NSEG % 2 == 0

            with tc.tile_pool(name=f"pps{layer_id}", bufs=2,
                              space="PSUM") as pps, \
                 tc.tile_pool(name=f"zps{layer_id}", bufs=2,
                              space="PSUM") as zps, \
                 tc.tile_pool(name=f"work{layer_id}", bufs=2) as work, \
                 tc.tile_pool(name=f"agp{layer_id}", bufs=1) as agpool, \
                 rep_loop():

                def emit_segment(seg, zx_read, zx_write, ctx_next):
                    """16 recurrence steps reading zx_read; proj m-tile j of
                    the NEXT segment (into zx_write) rides after step j."""
                    for j in range(CH):
                        hprev = h_ck[:, :, (j - 1) % CH, :]
                        # z viewed [128, gate, k, b]; m-tile = gate*4 + k
                        z_ps = zps.tile([128, 4, KH, BQ], F32, tag="zps")
                        # seed PSUM with zx via one identity matmul (opens
                        # the accumulation group); activations then read z
                        # straight from PSUM (no DVE add on the chain)
                        nc.tensor.matmul(
                            z_ps, id_sb, zx_read[:, :, ts(j, BQ)],
                            start=True, stop=False)
                        # k_in-major waves: wave k only needs h-chunk k of
                        # the previous step, which is produced in halves, so
                        # waves 0-1 start as soon as h-half0 lands. Within
                        # the last wave, the [f,i,g]-half0 tiles come first
                        # so the first gate ACT fires earliest.
                        worder = ([g * 4 + ko for g in (0, 1, 2)
                                   for ko in (0, 1)]
                                  + [g * 4 + ko for g in (0, 1, 2)
                                     for ko in (2, 3)]
                                  + [12, 13, 14, 15])
                        for k in range(KH):
                            for m in worder:
                                # stop only on the very last mm: a stop
                                # closes the whole 2KB psum zero-region, so
                                # per-tile stops would orphan later
                                # accumulations in the same bank.
                                nc.tensor.matmul(
                                    z_ps[:, m // 4, m % 4, :],
                                    u_sb[:, k, ts(m, 128)],
                                    hprev[:, k, :],
                                    start=False,
                                    stop=(k == KH - 1 and m == worder[-1]))
                        # gates host-permuted to [f, i, g, o] with i/f/o
                        # pre-activations halved on the host, so every gate
                        # is a plain tanh: tf=2f-1, ti=2i-1, tg=g, to=2o-1.
                        # Cell state tracked doubled (CS=2c); host halves the
                        # final output. Tail split into k-halves so h-half0
                        # unblocks the next step's first waves early.
                        g16 = work.tile([128, 4, KH, BQ], F32, tag="g16")
                        nc.scalar.activation(
                            g16[:, 0:3, :, :], z_ps[:, 0:3, :, :], AF.Tanh)
                        nc.scalar.activation(
                            g16[:, 3, :, :], z_ps[:, 3, :, :], AF.Tanh)
                        # fcS = (tf+1)*CS = 4fc ; ig2 = (ti+1)*g = 2ig
                        fc = work.tile([128, KH, BQ], F32, tag="fc")
                        nc.vector.scalar_tensor_tensor(
                            fc, g16[:, 0, :, :], 1.0, c_sb,
                            ALU.add, ALU.mult)
                        ig = work.tile([128, KH, BQ], F32, tag="ig")
                        nc.vector.scalar_tensor_tensor(
                            ig, g16[:, 1, :, :], 1.0, g16[:, 2, :, :],
                            ALU.add, ALU.mult)
                        # CS' = 2c' = fcS/2 + ig2
                        nc.vector.scalar_tensor_tensor(
                            c_sb, fc, 0.5, ig, ALU.mult, ALU.add)
                        th = work.tile([128, KH, BQ], F32, tag="th")
                        nc.scalar.activation(th, c_sb, AF.Tanh, scale=0.5)
                        # ring holds 2h = (to+1)*tanh(c)
                        nc.vector.scalar_tensor_tensor(
                            h_ck[:, :, j, :], g16[:, 3, :, :], 1.0,
                            th, ALU.add, ALU.mult)
                        # interleaved projection m-tile for the next segment
                        # (emitted last so the zx copy sits at the back of
                        # the per-step ACT queue, off the critical chain)
                        psum = pps.tile([128, TOKSEG], F32, tag="pps")
                        proj_mm(psum, j, ctx_next)
                        nc.scalar.activation(zx_write[:, j, :], psum,
                                             AF.Identity,
                                             bias=b_sb[:, j:j + 1])
                    # ship the segment's h ring to DRAM (off-engine)
                    ship(seg, h_ck)

                nc.vector.memset(c_sb, 0.0)
                nc.vector.memset(h_ck, 0.0)
                # prologue: projection of segment 0 into zxA
                ctx0 = seg_prep(0, agpool)
                for m in range(MC):
                    psum = pps.tile([128, TOKSEG], F32, tag="pps")
                    proj_mm(psum, m, ctx0)
                    nc.scalar.activation(zxA[:, m, :], psum, AF.Identity,
                                         bias=b_sb[:, m:m + 1])
                with tc.For_i(0, NSEG // 2, 1,
                              hint_engines=(mybir.EngineType.PE,)) as io:
                    from concourse.expressions import smin
                    zxs = (zxA, zxB)
                    for u in range(2):
                        segU = nc.s_assert_within(io * 2 + u, u,
                                                  NSEG - 2 + u)
                        segN = nc.s_assert_within(
                            smin(io * 2 + u + 1, NSEG - 1), 0, NSEG - 1)
                        ctxN = seg_prep(segN, agpool)
                        emit_segment(segU, zxs[u % 2], zxs[1 - u % 2], ctxN)
            state.release()

        # ---------------- layer 0 ----------------
        xpool = tc.alloc_tile_pool(name="xpool", bufs=1)
        x_sb = xpool.tile([128, KD0, TOK], BF16, name="x_sb")
        # split the 16MB x load so the first segment's staging can start
        # as soon as the first quarter lands
        for c in range(4):
            tq = TOK // 4
            nc.sync.dma_start(
                x_sb[:, :, ds(c * tq, tq)],
                xT.ap()[:, c * tq:(c + 1) * tq].rearrange(
                    "(k p) t -> p k t", p=128))
        w0pool = tc.alloc_tile_pool(name="w0pool", bufs=1)
        w0_sb = w0pool.tile([128, KD0, G], BF16, name="w0_sb")
        nc.sync.dma_start(w0_sb, w0.ap().rearrange("(k p) g -> p k g", p=128))
        u0_sb = w0pool.tile([128, KH, G], BF16, name="u0_sb")
        nc.sync.dma_start(u0_sb, u0.ap().rearrange("(k p) g -> p k g", p=128))

        def prep0(seg, agpool):
            # stage the segment's x slice statically so the 64 matmuls
            # carry no symbolic APs (PE register budget); Pool engine is
            # otherwise idle
            toff = nc.s_assert_within(seg * TOKSEG, 0, TOK - TOKSEG)
            xs = agpool.tile([128, KD0, TOKSEG], BF16, tag="xs")
            nc.gpsimd.tensor_copy(xs, x_sb[:, :, ds(toff, TOKSEG)])
            return xs

        def proj0(psum, m, xs):
            for k in range(KD0):
                nc.tensor.matmul(
                    psum, w0_sb[:, k, ts(m, 128)], xs[:, k, :],
                    start=(k == 0), stop=(k == KD0 - 1))

        def ship0(seg, ring):
            # per-segment ship straight from the ring: the exchange input
            # is complete the moment the last step finishes; a Pool copy
            # keeps the own-direction sequence resident in SBUF for L1
            toff = nc.s_assert_within(seg * TS, 0, T - TS)
            nc.scalar.dma_start(ag_in[:, :, ds(toff, TS), :], ring)
            nc.gpsimd.tensor_copy(h0_seq[:, :, ds(toff, TS), :], ring)

        layer(u0_sb, b0_sb, ship0, prep0, proj0, 0)
        w0pool.release()
        xpool.release()

        w1pool = tc.alloc_tile_pool(name="w1pool", bufs=1)
        w1_sb = w1pool.tile([128, 2 * KH, G], BF16, name="w1_sb")
        nc.sync.dma_start(w1_sb, w1.ap().rearrange("(k p) g -> p k g", p=128))
        u1_sb = w1pool.tile([128, KH, G], BF16, name="u1_sb")
        nc.sync.dma_start(u1_sb, u1.ap().rearrange("(k p) g -> p k g", p=128))

        if single_core:
            nc.sync.dma_start(ag_out[0], ag_in)
            nc.sync.dma_start(ag_out[1], ag_in)
        else:
            nc.gpsimd.collective_compute(
                "AllGather", ALU.bypass, replica_groups=groups,
                ins=[ag_in.opt()], outs=[ag_out.opt()])

        # ---------------- layer 1 ----------------
        pidx = nc.s_assert_within(1 - fv, 0, 1)

        def prep1(seg, agpool):
            # partner's storage segment mirrored in time, loaded straight
            agp = agpool.tile([128, KH, TS, BQ], BF16, tag="agp")
            src = ag_out[ds(pidx, 1), :, :,
                         ds((NSEG - 1) * TS - seg * TS, TS), :]
            nc.sync.dma_start(agp, src.squeeze(0))
            # stage own-direction h statically (PE register budget); Pool
            # engine is otherwise idle
            toff = nc.s_assert_within(seg * TS, 0, T - TS)
            own = agpool.tile([128, KH, TS, BQ], BF16, tag="own")
            nc.gpsimd.tensor_copy(own, h0_seq[:, :, ds(toff, TS), :])
            return (agp, own)

        def proj1(psum, m, ctx):
            agp, own = ctx
            # own-direction features (straight)
            for k in range(KH):
                nc.tensor.matmul(
                    psum, w1_sb[:, k, ts(m, 128)], own[:, k, :, :],
                    start=(k == 0), stop=False)
            # partner features: reversed within the loaded tile
            for k in range(KH):
                nc.tensor.matmul(
                    psum, w1_sb[:, KH + k, ts(m, 128)],
                    agp[:, k, ::-1, :],
                    start=False, stop=(k == KH - 1))

        def ship1(seg, ring):
            # issued from the ACT queue: the SP engine's register pool is
            # exhausted by the partner-segment DMAs in the same loop body
            toff = nc.s_assert_within(seg * TS, 0, T - TS)
            nc.scalar.dma_start(h1T.ap()[:, :, ds(toff, TS), :], ring)

        layer(u1_sb, b1_sb, ship1, prep1, proj1, 1)
        w1pool.release()

        seqs.release()
        consts.release()
        dram.release()

    _fix_walrus_compat(nc)
    return nc


def _prep_core_inputs(x, W0f, U0f, b0f, W0b, U0b, b0b,
                      W1f, U1f, b1f, W1b, U1b, b1b, T, BQ):
    """Host-side sharding: returns list of 8 input dicts (core = 2q+dir).

    Backward cores receive time-reversed x and W1 with row halves swapped
    (own-direction features first)."""
    bf = ml_dtypes.bfloat16
    H = U0f.shape[0]

    def gperm(M, row_scale=1.0):
        # reorder gate blocks [i, f, g, o] -> [f, i, g, o] on the last axis,
        # halve the f/i/o pre-activations (sigmoid(x) = (tanh(x/2)+1)/2 so
        # the device computes every gate as a plain tanh), and apply
        # row_scale (0.5 for matrices consuming the doubled h convention).
        M = np.asarray(M, dtype=np.float32)
        M4 = M.reshape(*M.shape[:-1], 4, H)[..., [1, 0, 2, 3], :]
        cs = np.array([0.5, 0.5, 1.0, 0.5], dtype=np.float32)
        M4 = M4 * cs[:, None] * np.float32(row_scale)
        return np.ascontiguousarray(M4.reshape(M.shape))

    in_maps = []
    Wd = {0: (W0f, U0f, b0f, W1f, U1f, b1f),
          1: (W0b, U0b, b0b, W1b, U1b, b1b)}
    for q in range(N_Q):
        xq = x[q * BQ:(q + 1) * BQ]              # [BQ, T, D]
        for d in range(2):
            W0, U0, b0, W1, U1, b1 = Wd[d]
            xd = xq if d == 0 else xq[:, ::-1]
            xTc = np.ascontiguousarray(
                xd.transpose(2, 1, 0).reshape(x.shape[2], T * BQ)).astype(bf)
            W1c = W1 if d == 0 else np.concatenate([W1[H:], W1[:H]], axis=0)
            in_maps.append({
                "xT": xTc,
                "w0": gperm(W0).astype(bf),
                "u0": gperm(U0, 0.5).astype(bf),
                "b0": gperm(b0).astype(np.float32),
                "w1": gperm(W1c, 0.5).astype(bf),
                "u1": gperm(U1, 0.5).astype(bf),
                "b1": gperm(b1).astype(np.float32),
                "flag": np.array([[d]], dtype=np.int32),
                "ident": np.eye(128, dtype=np.float32).astype(bf),
            })
    return in_maps


def _unshard(results, B, T, H, BQ):
    out = np.empty((B, T, 2 * H), dtype=np.float32)
    KH = H // 128
    for q in range(N_Q):
        for d in range(2):
            h1 = np.asarray(results[2 * q + d]["h1T"], dtype=np.float32)
            # h1 [128, KH, T, BQ] holds 2*h (doubled-h convention)
            h = h1.transpose(3, 2, 1, 0).reshape(BQ, T, H) * np.float32(0.5)
            if d == 1:
                h = h[:, ::-1]
            out[q * BQ:(q + 1) * BQ, :, d * H:(d + 1) * H] = h
    return out


def kernel(x, W0f, U0f, b0f, W0b, U0b, b0b,
           W1f, U1f, b1f, W1b, U1b, b1b):
    x = np.asarray(x, dtype=np.float32)
    B, T, D = x.shape
    H = U0f.shape[0]
    BQ = B // N_Q
    nc = build_program(T=T, BQ=BQ, D=D, H=H)
    in_maps = _prep_core_inputs(
        np.asarray(x), np.asarray(W0f), np.asarray(U0f), np.asarray(b0f),
        np.asarray(W0b), np.asarray(U0b), np.asarray(b0b),
        np.asarray(W1f), np.asarray(U1f), np.asarray(b1f),
        np.asarray(W1b), np.asarray(U1b), np.asarray(b1b), T, BQ)
    res = run_bass_kernel_spmd(nc, in_maps, list(range(N_CORES)))
    return _unshard(res.results, B, T, H, BQ)

